# revision 1
# baseline (speedup 1.0000x reference)
"""Trainium2 Bass kernel for nn_Bottleneck_MDTA (B=16, C=256, H=W=64, heads=4).

Data-parallel over batch: 16 samples -> 8 cores x 2 samples. All conv /
attention weights replicated per core. Channel-major on-chip layout
[C on partitions, spatial on free dim], bf16 matmul operands, fp32 PSUM
accumulation, fp32 residual path.

Pipeline per sample:
  x --(conv3x3+BN+SiLU)--> y1 --(conv3x3+BN+SiLU)--> y2 --(1x1 conv)--> qkv
  qkv --(depthwise 3x3 via PE diagonal matmuls)--> q,k,v
  q,k: L2-normalized over spatial (norm stats on DVE, fold into softmax/k)
  attn = softmax(temp * q_hat @ k_hat^T) per 64-channel head (chan attention)
  out = x + proj(attn @ v)
"""

import numpy as np
import ml_dtypes

import concourse.bass as bass
import concourse.tile as tile
from concourse import bacc, mybir
from concourse.bass_utils import run_bass_kernel_spmd

BF = mybir.dt.bfloat16
F32 = mybir.dt.float32
AF = mybir.ActivationFunctionType
OP = mybir.AluOpType
AX = mybir.AxisListType

N_CORES = 8
S = 2            # samples per core
CB = 2           # channel blocks of 128 (C=256)
QKVB = 6         # qkv channel blocks (768)
P = 128
H = W = 64
HW = H * W
WS = 68          # padded row stride (W + 2 pad + 2 align)
C0 = 4           # interior column offset (left pad)
R0 = 1           # interior row offset (top pad)
NROWS = 67       # 66 real rows + 1 spare for flat-slice slack
PS = NROWS * WS  # padded buffer free size (4556)
NT = 8           # spatial h-tiles per sample-block (8 rows x 64 cols = 512)
EPS_NORM = 1e-12

_CACHE = {}


def _pad_read(buf2d, h0, dy, dx):
    """rhs AP for a conv tap: rows h0+dy .. h0+dy+7, cols shifted by dx-1.

    buf2d: [128, PS] padded AP. Returns [128, 8, 64]. Out-of-row reads wrap
    into the next row's left pad, which is kept zero (correct conv padding).
    """
    st = (h0 + dy) * WS + (C0 - 1) + dx
    return buf2d[:, st:st + 8 * WS].rearrange("p (r c) -> p r c", c=WS)[:, :, 0:64]


def _interior(buf2d, h0):
    """Write AP for interior rows h0..h0+7 of a padded buffer: [128, 8, 64]."""
    st = (h0 + R0) * WS + C0
    return buf2d[:, st:st + 8 * WS].rearrange("p (r c) -> p r c", c=WS)[:, :, 0:64]


def _memset_pads(nc, t2d):
    """Zero the pad regions of a padded [128, PS] tile."""
    # top pad row 0
    nc.gpsimd.memset(t2d[:, 0:WS], 0.0)
    # bottom pad rows 65, 66
    nc.gpsimd.memset(t2d[:, 65 * WS:67 * WS], 0.0)
    # left pad cols 0..3 of rows 1..64
    strip = t2d[:, WS:65 * WS].rearrange("p (r c) -> p r c", c=WS)[:, :, 0:C0]
    nc.gpsimd.memset(strip, 0.0)


def build_bass():
    nc = bacc.Bacc("TRN2", target_bir_lowering=False, debug=False,
                   num_devices=N_CORES)

    # ---- DRAM I/O ----
    x_d = nc.dram_tensor("x", [S, 256, H, W], F32, kind="ExternalInput").ap()
    w1_d = nc.dram_tensor("w1", [9, CB, P, P], BF, kind="ExternalInput").ap()
    b1_d = nc.dram_tensor("b1", [P, 1], F32, kind="ExternalInput").ap()
    w2_d = nc.dram_tensor("w2", [9, CB, P, P], BF, kind="ExternalInput").ap()
    b2_d = nc.dram_tensor("b2", [P, CB], F32, kind="ExternalInput").ap()
    wq_d = nc.dram_tensor("wq", [CB, QKVB, P, P], BF, kind="ExternalInput").ap()
    dwv_d = nc.dram_tensor("dwv", [9, QKVB, P, P], BF, kind="ExternalInput").ap()
    dws_d = nc.dram_tensor("dws", [P, 9, QKVB], F32, kind="ExternalInput").ap()
    wp_d = nc.dram_tensor("wp", [CB, CB, P, P], BF, kind="ExternalInput").ap()
    tmp_d = nc.dram_tensor("tmp", [P, CB], F32, kind="ExternalInput").ap()
    idn_d = nc.dram_tensor("idn", [P, P], BF, kind="ExternalInput").ap()
    out_d = nc.dram_tensor("out", [S, 256, H, W], F32, kind="ExternalOutput").ap()

    # ---- persistent SBUF ----
    w1s = nc.alloc_sbuf_tensor("w1s", [P, 9, CB, P], BF).ap()
    w2s = nc.alloc_sbuf_tensor("w2s", [P, 9, CB, P], BF).ap()
    wqs = nc.alloc_sbuf_tensor("wqs", [P, CB, QKVB, P], BF).ap()
    wps = nc.alloc_sbuf_tensor("wps", [P, CB, CB, P], BF).ap()
    idns = nc.alloc_sbuf_tensor("idns", [P, P], BF).ap()
    b1s = nc.alloc_sbuf_tensor("b1s", [P, 1], F32).ap()
    b2s = nc.alloc_sbuf_tensor("b2s", [P, CB], F32).ap()
    tmps = nc.alloc_sbuf_tensor("tmps", [P, CB], F32).ap()
    dwss = nc.alloc_sbuf_tensor("dwss", [P, 9, QKVB], F32).ap()

    kd = nc.alloc_sbuf_tensor("kd", [P, CB, HW], BF).ap()
    vd = nc.alloc_sbuf_tensor("vd", [P, CB, HW], BF).ap()
    qT = nc.alloc_sbuf_tensor("qT", [P, CB, HW], BF).ap()
    kT = nc.alloc_sbuf_tensor("kT", [P, CB, HW], BF).ap()
    attnE = nc.alloc_sbuf_tensor("attnE", [P, CB, P], BF).ap()
    attnTs = nc.alloc_sbuf_tensor("attnTs", [P, CB, P], BF).ap()

    # small fp32 stats: per block columns
    qsqp = nc.alloc_sbuf_tensor("qsqp", [P, CB, NT], F32).ap()
    ksqp = nc.alloc_sbuf_tensor("ksqp", [P, CB, NT], F32).ap()
    qsq = nc.alloc_sbuf_tensor("qsq", [P, CB], F32).ap()
    ksq = nc.alloc_sbuf_tensor("ksq", [P, CB], F32).ap()
    invq = nc.alloc_sbuf_tensor("invq", [P, CB], F32).ap()
    invk = nc.alloc_sbuf_tensor("invk", [P, CB], F32).ap()
    srow = nc.alloc_sbuf_tensor("srow", [P, CB], F32).ap()
    zacc = nc.alloc_sbuf_tensor("zacc", [P, CB], F32).ap()
    zrec = nc.alloc_sbuf_tensor("zrec", [P, CB], F32).ap()
    dumq = nc.alloc_sbuf_tensor("dumq", [P, 1], BF).ap()

    with tile.TileContext(nc) as tc:
        with (
            tc.tile_pool(name="big", bufs=2) as bigp,          # xpad/y1/y2
            tc.tile_pool(name="qkvp", bufs=2) as qkvp,         # padded qkv blk
            tc.tile_pool(name="xcv", bufs=2) as xcvp,          # x f32 stream
            tc.tile_pool(name="xrs", bufs=3) as xrsp,          # residual x f32
            tc.tile_pool(name="qdp", bufs=3) as qdp,           # q evac chunks
            tc.tile_pool(name="sgp", bufs=3) as sgp,           # sigmoid scratch
            tc.tile_pool(name="aop", bufs=4) as aopp,          # attn-out chunks
            tc.tile_pool(name="osb", bufs=3) as osbp,          # out f32 chunks
            tc.tile_pool(name="dwp", bufs=10) as dwp,           # diag dw tiles
            tc.tile_pool(name="accp", bufs=2) as accp,         # dw DVE accum
            tc.tile_pool(name="shfp", bufs=1) as shfp,         # shifted qkv copy
            tc.tile_pool(name="ps512", bufs=5, space="PSUM") as ps512,
            tc.tile_pool(name="psA", bufs=2, space="PSUM") as psA,
            tc.tile_pool(name="psT", bufs=1, space="PSUM") as psT,
        ):
            # ---- preamble: cv1 weights on the sync queue (x-loads follow
            # there); everything needed later goes on the vector/scalar DMA
            # queues so the first conv isn't serialized behind them ----
            nc.sync.dma_start(w1s, w1_d.rearrange("t b i o -> i t b o"))
            nc.sync.dma_start(b1s, b1_d)
            nc.gpsimd.memset(attnE[:, :, :], 0.0)

            def emit_weight_preloads():
                # deferred weight preloads on the gpsimd SWDGE queue --
                # emitted after sample 0's pad-memsets so those aren't
                # queued behind ~2.5MB of weight traffic at startup
                nc.gpsimd.dma_start(w2s, w2_d.rearrange("t b i o -> i t b o"))
                nc.gpsimd.dma_start(wqs, wq_d.rearrange("b q i o -> i b q o"))
                nc.gpsimd.dma_start(wps, wp_d.rearrange("b q i o -> i b q o"))
                nc.gpsimd.dma_start(idns, idn_d)
                nc.gpsimd.dma_start(b2s, b2_d)
                nc.gpsimd.dma_start(tmps, tmp_d)
                nc.gpsimd.dma_start(dwss, dws_d)

            for s in range(S):
                # ================= stage A: load x, convert to bf16 =========
                xpad_t = bigp.tile([P, CB, PS], BF, tag="act")
                for cb in range(CB):
                    _memset_pads(nc, xpad_t[:, cb, :])
                for h0 in range(0, H, 8):
                    for cb in range(CB):
                        xc = xcvp.tile([P, 8, 64], F32)
                        nc.sync.dma_start(
                            xc, x_d[s, cb * P:(cb + 1) * P, h0:h0 + 8, :])
                        nc.vector.tensor_copy(
                            _interior(xpad_t[:, cb, :], h0), xc)

                if s == 0:
                    emit_weight_preloads()

                # ================= stage B: cv1 (+BN+SiLU) ==================
                y1_t = bigp.tile([P, 1, PS], BF, tag="act")
                _memset_pads(nc, y1_t[:, 0, :])
                for h0 in range(0, H, 8):
                    ps = ps512.tile([P, 512], F32, tag="ps")
                    n = 0
                    for t in range(9):
                        dy, dx = t // 3, t % 3
                        for cb in range(CB):
                            nc.tensor.matmul(
                                ps, w1s[:, t, cb, :],
                                _pad_read(xpad_t[:, cb, :], h0, dy, dx),
                                start=(n == 0), stop=(n == 17))
                            n += 1
                    sg = sgp.tile([P, 512], BF, tag="sg")
                    nc.scalar.activation(sg, ps, AF.Sigmoid, bias=b1s)
                    nc.vector.scalar_tensor_tensor(
                        _interior(y1_t[:, 0, :], h0), ps, b1s, sg,
                        op0=OP.add, op1=OP.mult)

                # ================= stage C: cv2 (+BN+SiLU) ==================
                y2_t = bigp.tile([P, CB, PS], BF, tag="act")
                for cb in range(CB):
                    _memset_pads(nc, y2_t[:, cb, :])
                for co in range(CB):
                    for h0 in range(0, H, 8):
                        ps = ps512.tile([P, 512], F32, tag="ps")
                        for t in range(9):
                            dy, dx = t // 3, t % 3
                            nc.tensor.matmul(
                                ps, w2s[:, t, co, :],
                                _pad_read(y1_t[:, 0, :], h0, dy, dx),
                                start=(t == 0), stop=(t == 8))
                        sg = sgp.tile([P, 512], BF, tag="sg")
                        nc.scalar.activation(
                            sg, ps, AF.Sigmoid, bias=b2s[:, co:co + 1])
                        nc.vector.scalar_tensor_tensor(
                            _interior(y2_t[:, co, :], h0), ps,
                            b2s[:, co:co + 1], sg, op0=OP.add, op1=OP.mult)

                # ========= stage D: qkv 1x1 conv + depthwise 3x3 ============
                # Block order k(2,3) -> q(0,1) -> v(4,5), software-pipelined:
                # conv of block b+1 is emitted between conv(b) and dw(b) so
                # the PE stream has fill work while ACT evacuates qk_t.
                # Per-k-block norm finalize + kd scale + kT transposes are
                # emitted eagerly so they overlap later blocks; attention
                # scores are emitted before the v blocks.
                # per-kind tap split: v blocks (processed last) get a
                # shorter DVE chain so attn@v is not stalled on the DVE tail
                DVE_TAPS_QK = [(1, 1), (0, 1), (2, 1), (0, 0), (1, 0)]
                PE_TAPS_QK = [(2, 0), (0, 2), (1, 2), (2, 2)]
                DVE_TAPS_V = [(1, 1), (0, 1), (2, 1)]
                PE_TAPS_V = [(2, 0), (0, 0), (1, 0), (0, 2), (1, 2), (2, 2)]
                TAPS = {0: (DVE_TAPS_QK, PE_TAPS_QK),
                        1: (DVE_TAPS_QK, PE_TAPS_QK),
                        2: (DVE_TAPS_QK, PE_TAPS_QK),
                        3: (DVE_TAPS_QK, PE_TAPS_QK),
                        4: (DVE_TAPS_V, PE_TAPS_V),
                        5: (DVE_TAPS_V, PE_TAPS_V)}
                ALEN = 4352   # acc length: padded rows 1..64 (i = m - 68)

                def _acci(buf, h0):
                    return buf[:, h0 * WS + C0:h0 * WS + C0 + 8 * WS] \
                        .rearrange("p (r c) -> p r c", c=WS)[:, :, 0:64]

                def emit_conv(qb):
                    pe_taps = TAPS[qb][1]
                    dwt = []
                    for t3 in range(len(pe_taps)):
                        dy2, dx2 = pe_taps[t3]
                        d = dwp.tile([P, P], BF, tag="dw", name=f"dw{t3}")
                        nc.sync.dma_start(d, dwv_d[dy2 * 3 + dx2, qb, :, :])
                        dwt.append(d)
                    qk_t = qkvp.tile([P, PS], BF, name="qk_t")
                    _memset_pads(nc, qk_t)
                    for h0 in range(0, H, 8):
                        ps = ps512.tile([P, 512], F32, tag="ps", name="psc")
                        for cb in range(CB):
                            nc.tensor.matmul(
                                ps, wqs[:, cb, qb, :],
                                _interior(y2_t[:, cb, :], h0),
                                start=(cb == 0), stop=(cb == 1))
                        nc.scalar.copy(_interior(qk_t, h0), ps)
                    return (qb, qk_t, dwt)

                def emit_dw(state):
                    qb, qk_t, dwt = state
                    kind, cb = qb // 2, qb % 2
                    dve_taps, pe_taps = TAPS[qb]
                    # DVE accumulator: two half-buffer chains (rows 1-32 /
                    # 33-64) so the first materialize chunks + transposes can
                    # start after half a chain instead of the whole one.
                    acc = accp.tile([P, ALEN + 8], BF, name="acc")
                    if any(dx == 0 for _, dx in dve_taps):
                        shf = shfp.tile([P, ALEN], BF, name="shf")
                        nc.gpsimd.tensor_copy(
                            shf[:, 0:2244], qk_t[:, 67:67 + 2244])
                        nc.gpsimd.tensor_copy(
                            shf[:, 2244:ALEN], qk_t[:, 67 + 2244:67 + ALEN])
                    def emit_half(half):
                        hlo = half * (ALEN // 2)
                        hhi = (half + 1) * (ALEN // 2)
                        first = True
                        for dy, dx in dve_taps:
                            wsc = dwss[:, dy * 3 + dx, qb:qb + 1]
                            i0 = max(68 if dy == 0 else 0, hlo)
                            i1 = min(ALEN - 68 if dy == 2 else ALEN, hhi)
                            if i0 >= i1:
                                continue
                            if dx == 1:
                                src = qk_t[:, i0 + 68 * dy:i1 + 68 * dy]
                            else:
                                src = shf[:, i0 + 68 * dy - 68:
                                          i1 + 68 * dy - 68]
                            if first:
                                nc.vector.tensor_scalar_mul(
                                    acc[:, i0:i1], src, wsc)
                                first = False
                            else:
                                tmp = shfp.tile([P, ALEN], BF, tag="tmp",
                                                name="tmp")
                                nc.vector.tensor_scalar_mul(
                                    tmp[:, i0:i1], src, wsc)
                                nc.vector.tensor_add(
                                    acc[:, i0:i1], acc[:, i0:i1],
                                    tmp[:, i0:i1])
                    # PE taps into PSUM, ACT-evacuated to pb; final chunk =
                    # acc + pb on DVE (psum released immediately, PE never
                    # blocks behind the DVE chain)
                    pb = accp.tile([P, ALEN + 8], BF, tag="pb", bufs=1,
                                   name="pb")
                    emit_half(0)
                    emit_half(1)
                    for h0 in range(0, H, 8):
                        j = h0 // 8
                        ps = ps512.tile([P, 512], F32, tag="ps", name="psd")
                        for t3 in range(len(pe_taps)):
                            dy, dx = pe_taps[t3]
                            nc.tensor.matmul(
                                ps, dwt[t3], _pad_read(qk_t, h0, dy, dx),
                                start=(t3 == 0), stop=(t3 == len(pe_taps) - 1))
                        nc.scalar.copy(_acci(pb, h0), ps)
                        if kind == 0:    # q: add partials, norms, transpose
                            qc = qdp.tile([P, 512], BF, name="qc")
                            nc.vector.tensor_add(
                                qc.rearrange("p (r c) -> p r c", c=64),
                                _acci(acc, h0), _acci(pb, h0))
                            sq = sgp.tile([P, 512], BF, tag="sq", name="sq")
                            nc.scalar.activation(
                                sq, qc, AF.Square,
                                accum_out=qsqp[:, cb, j:j + 1])
                            nc.sync.dma_start_transpose(
                                qT[:, cb, j * 512:(j + 1) * 512]
                                .rearrange("p (a b) -> p a b", b=P), qc)
                        elif kind == 1:  # k: into kd (scaled later)
                            kc = kd[:, cb, j * 512:(j + 1) * 512]
                            nc.vector.tensor_add(
                                kc.rearrange("p (r c) -> p r c", c=64),
                                _acci(acc, h0), _acci(pb, h0))
                            sq = sgp.tile([P, 512], BF, tag="sq", name="sq")
                            nc.scalar.activation(
                                sq, kc, AF.Square,
                                accum_out=ksqp[:, cb, j:j + 1])
                        else:            # v: into vd
                            nc.vector.tensor_add(
                                vd[:, cb, j * 512:(j + 1) * 512]
                                .rearrange("p (r c) -> p r c", c=64),
                                _acci(acc, h0), _acci(pb, h0))

                def emit_kfin(cb):
                    nc.vector.tensor_reduce(
                        ksq[:, cb:cb + 1], ksqp[:, cb, :], axis=AX.X,
                        op=OP.add)
                    nc.scalar.sqrt(ksq[:, cb:cb + 1], ksq[:, cb:cb + 1])
                    nc.vector.tensor_scalar_max(
                        ksq[:, cb:cb + 1], ksq[:, cb:cb + 1], EPS_NORM)
                    nc.vector.reciprocal(invk[:, cb:cb + 1], ksq[:, cb:cb + 1])
                    nc.vector.tensor_scalar_mul(
                        kd[:, cb, :], kd[:, cb, :], invk[:, cb:cb + 1])
                    for j in range(NT):
                        nc.sync.dma_start_transpose(
                            kT[:, cb, j * 512:(j + 1) * 512]
                            .rearrange("p (a b) -> p a b", b=P),
                            kd[:, cb, j * 512:(j + 1) * 512])

                def emit_qfin(cb):
                    nc.vector.tensor_reduce(
                        qsq[:, cb:cb + 1], qsqp[:, cb, :], axis=AX.X,
                        op=OP.add)
                    nc.scalar.sqrt(qsq[:, cb:cb + 1], qsq[:, cb:cb + 1])
                    nc.vector.tensor_scalar_max(
                        qsq[:, cb:cb + 1], qsq[:, cb:cb + 1], EPS_NORM)
                    nc.vector.reciprocal(invq[:, cb:cb + 1], qsq[:, cb:cb + 1])
                    nc.vector.tensor_tensor(
                        srow[:, cb:cb + 1], tmps[:, cb:cb + 1],
                        invq[:, cb:cb + 1], OP.mult)

                def emit_attn_scores():
                    # all score matmuls first (PE stream stays full while the
                    # softmax/transpose tail waits on srow/exp), then the
                    # per-pair exp -> transpose -> 1/Z tail
                    a0s = []
                    for pb2 in range(CB):   # head pair pb2: heads 2p,2p+1
                        a0 = psA.tile([P, P], F32, name=f"a0{pb2}", tag="a0")
                        for jj in range(HW // P):
                            nc.tensor.matmul(
                                a0, qT[:, pb2, jj * P:(jj + 1) * P],
                                kT[:, pb2, jj * P:(jj + 1) * P],
                                start=(jj == 0), stop=(jj == HW // P - 1))
                        a0s.append(a0)
                    for pb2 in range(CB):
                        for hh in range(2):
                            sl = slice(hh * 64, (hh + 1) * 64)
                            nc.scalar.activation(
                                attnE[sl, pb2, sl], a0s[pb2][sl, sl], AF.Exp,
                                scale=srow[sl, pb2:pb2 + 1],
                                accum_out=zacc[sl, pb2:pb2 + 1])
                        at = psT.tile([P, P], BF, name="at")
                        nc.tensor.transpose(at, attnE[:, pb2, :], idns)
                        nc.scalar.copy(attnTs[:, pb2, :], at)
                        nc.vector.reciprocal(
                            zrec[:, pb2:pb2 + 1], zacc[:, pb2:pb2 + 1])

                ORDER = [2, 3, 0, 1, 4, 5]
                POST = {2: lambda: emit_kfin(0), 3: lambda: emit_kfin(1),
                        0: lambda: emit_qfin(0),
                        1: lambda: (emit_qfin(1), emit_attn_scores())}
                pending = None
                for qb in ORDER:
                    st = emit_conv(qb)
                    if pending is not None:
                        emit_dw(pending)
                        hook = POST.get(pending[0])
                        if hook:
                            hook()
                    pending = st
                emit_dw(pending)
                hook = POST.get(pending[0])
                if hook:
                    hook()

                # ============ stage G+H: attn@v, proj, residual =============
                for j in range(NT):
                    aot = []
                    for pb in range(CB):
                        ps = ps512.tile([P, 512], F32, tag="ps")
                        for hh in range(2):
                            sl = slice(hh * 64, (hh + 1) * 64)
                            nc.tensor.matmul(
                                ps[sl, :],
                                attnTs[sl, pb, hh * 64:(hh + 1) * 64],
                                vd[sl, pb, j * 512:(j + 1) * 512])
                        ao = aopp.tile([P, 512], BF)
                        nc.scalar.mul(ao, ps, zrec[:, pb:pb + 1])
                        aot.append(ao)
                    for co in range(CB):
                        ps = ps512.tile([P, 512], F32, tag="ps")
                        for cb in range(CB):
                            nc.tensor.matmul(
                                ps, wps[:, cb, co, :], aot[cb],
                                start=(cb == 0), stop=(cb == 1))
                        xr = xrsp.tile([P, 8, 64], F32)
                        nc.sync.dma_start(
                            xr, x_d[s, co * P:(co + 1) * P, j * 8:j * 8 + 8, :])
                        ot = osbp.tile([P, 8, 64], F32)
                        nc.vector.tensor_tensor(
                            ot, ps.rearrange("p (r c) -> p r c", c=64),
                            xr, OP.add)
                        nc.sync.dma_start(
                            out_d[s, co * P:(co + 1) * P, j * 8:j * 8 + 8, :],
                            ot)

    nc.compile()
    return nc


def prep_inputs(inputs):
    """Host-side: fold BN, reshape weights into lhsT tiles, cast to bf16."""
    f = {k: np.asarray(v, dtype=np.float32) for k, v in inputs.items()}
    bf16 = ml_dtypes.bfloat16

    def taps(wfold, blocks_in, blocks_out, per_out_block):
        # wfold: [Cout, Cin, 3, 3] -> [9, nblk, 128, 128] lhsT tiles [ci, co]
        outb = []
        for t in range(9):
            dy, dx = t // 3, t % 3
            row = []
            if per_out_block:
                for ob in range(blocks_out):
                    blk = wfold[ob * P:(ob + 1) * P, :, dy, dx]  # [co, ci]
                    row.append(np.ascontiguousarray(blk.T))
            else:
                for ib in range(blocks_in):
                    blk = wfold[:, ib * P:(ib + 1) * P, dy, dx]
                    row.append(np.ascontiguousarray(blk.T))
            outb.append(row)
        return np.asarray(outb, dtype=bf16)

    s1 = f["cv1_g"] / np.sqrt(f["cv1_v"] + 1e-5)
    w1f = f["cv1_w"] * s1[:, None, None, None]
    b1 = (f["cv1_b"] - f["cv1_m"] * s1).reshape(P, 1).astype(np.float32)
    w1 = taps(w1f, 2, 1, per_out_block=False)          # [9, 2, 128, 128]

    s2 = f["cv2_g"] / np.sqrt(f["cv2_v"] + 1e-5)
    w2f = f["cv2_w"] * s2[:, None, None, None]
    b2v = f["cv2_b"] - f["cv2_m"] * s2
    b2 = np.ascontiguousarray(b2v.reshape(CB, P).T).astype(np.float32)
    w2 = taps(w2f, 1, 2, per_out_block=True)           # [9, 2, 128, 128]

    wqf = f["qkv_w"][:, :, 0, 0]                       # [768, 256]
    wq = np.zeros((CB, QKVB, P, P), dtype=bf16)
    for ib in range(CB):
        for ob in range(QKVB):
            wq[ib, ob] = wqf[ob * P:(ob + 1) * P, ib * P:(ib + 1) * P].T

    dwf = f["dw_w"][:, 0, :, :]                        # [768, 3, 3]
    dwv = np.zeros((9, QKVB, P, P), dtype=bf16)
    for t in range(9):
        dy, dx = t // 3, t % 3
        for qb in range(QKVB):
            dwv[t, qb] = np.diag(dwf[qb * P:(qb + 1) * P, dy, dx])
    dws = np.zeros((P, 9, QKVB), dtype=np.float32)
    for t in range(9):
        dy, dx = t // 3, t % 3
        for qb in range(QKVB):
            dws[:, t, qb] = dwf[qb * P:(qb + 1) * P, dy, dx]

    wpf = f["proj_w"][:, :, 0, 0]
    wp = np.zeros((CB, CB, P, P), dtype=bf16)
    for ib in range(CB):
        for ob in range(CB):
            wp[ib, ob] = wpf[ob * P:(ob + 1) * P, ib * P:(ib + 1) * P].T

    temp = f["temperature"].reshape(4)
    tmp = np.zeros((P, CB), dtype=np.float32)
    for cb in range(CB):
        for p in range(P):
            tmp[p, cb] = temp[(cb * P + p) // 64]

    return {
        "w1": w1, "b1": b1, "w2": w2, "b2": b2, "wq": wq, "dwv": dwv,
        "dws": dws, "wp": wp, "tmp": tmp, "idn": np.eye(P, dtype=bf16),
    }


def get_nc():
    if "nc" not in _CACHE:
        _CACHE["nc"] = build_bass()
    return _CACHE["nc"]


def kernel(**inputs):
    nc = get_nc()
    shared = prep_inputs(inputs)
    x = np.asarray(inputs["x"], dtype=np.float32)
    in_maps = []
    for c in range(N_CORES):
        m = dict(shared)
        m["x"] = np.ascontiguousarray(x[c * S:(c + 1) * S])
        in_maps.append(m)
    res = run_bass_kernel_spmd(nc, in_maps, core_ids=list(range(N_CORES)))
    out = np.concatenate([res.results[c]["out"] for c in range(N_CORES)], axis=0)
    return out.astype(np.float32)

